# revision 3
# baseline (speedup 1.0000x reference)
"""Trainium2 Bass kernel for nn_Block_27384711479862 (ConvNeXt-ish metaformer block).

Per-core computation (data parallel over batch B=8 -> 8 cores), fp8e4m3
everywhere in the branches (branch outputs are scaled by ls1=ls2=0.01, so
fp8 branch error is damped 100x vs the f32 residual path), with DoubleRow
(dual-fp8) matmuls processing two 128-deep k-tiles per pass:

  x: [C=384, N=2304]  (N = 48*48 spatial), residual path kept in f32.
  attention (transposed-scores scheme, no PE transpose needed):
      q8 = 8*(qw@x)+8qb, k8 likewise         (fp8 DR matmuls, fp8 out)
      sT[m,n] = k8[:,m].q8[:,n] = 64*s       (DR over 4 c-chunks, pad=0)
      aT[m,n] = exp(sT/(64*sqrt(C)) - 2.5)   (ACT, fp8 out, <240 cap ok)
      l[n]    = ones.aT  (DR ones-stationary M=32 matmul, PSUM [32,n])
      vT[m,c] = 8*x.T@vw                     (DR)
      rb      = bcast(0.125/l)  (DVE recip + K=1 ones matmul)
      attn    = (vT.T @ aT) * rb             (DR + DVE evict = softmax@v)
      x1' = 32*x1 = (32*al1).x + (32*pw).attn   (stt evict; beta1 deferred)
  mlp:
      h = fc1(x1q)/2^8 + f1b   (x1q = fp8(x1'), f1w scaled 8x; padded 50x50)
      dw = 3x3 depthwise: 4 DR tap-pairs (dy-pairs stride 50, (6,8) stride 2)
           + 1 plain fp8 tap, PSUM accumulated; g = gelu(dw + dwb) fp8
      out = (x1'/32) + (fc2_64(g)/2^6 + resb)   (ACT + DVE stt evict)
"""
import numpy as np
import ml_dtypes

C = 384
HID = 1536
H = W = 48
N = H * W              # 2304
NC_ = 3                # C chunks of 128
NCP = 4                # padded C chunks (chunk 3 = zeros)
NH = 12                # HID chunks of 128
NMB = 18               # key-position blocks of 128
EPS = 1e-5
BF16 = ml_dtypes.bfloat16
F8 = ml_dtypes.float8_e4m3

NT5 = [(i * 512, min(512, N - i * 512)) for i in range((N + 511) // 512)]
NT6 = [(i * 384, 384) for i in range(6)]
ROWS_PER_TILE = 8
PAD = 50

# dwconv tap pairing: pairs with legal (>=2) elem strides in the padded
# [50,50] layout; (t,t+3) pairs differ by one row (stride 50), (6,8) by
# two cols (stride 2); tap 7 is the plain-matmul single.
DW_PAIRS = [(0, 3, 50), (1, 4, 50), (2, 5, 50), (6, 8, 2)]
DW_SINGLE = 7
DW_ORDER = [0, 3, 1, 4, 2, 5, 6, 8, 7]   # host diag-block order

ESC = float(1.0 / (64.0 * np.sqrt(np.float32(C))))
EBIAS = -2.5

_PROG = None


def _pairw(base_ap, stride):
    """Add a DR pair dim (size 2, given elem stride) in front of free dims."""
    shape = (base_ap.shape[0], 2) + tuple(base_ap.shape[1:])
    u = base_ap.unsqueeze(1).to_broadcast(shape)
    apl = u.ap
    apl[1] = [stride, 2]
    u.ap = apl
    return u


def _build_program(iters=1):
    import concourse.bacc as bacc
    import concourse.mybir as mybir
    import concourse.tile as tile
    from contextlib import ExitStack

    dt = mybir.dt
    AF = mybir.ActivationFunctionType
    ALU = mybir.AluOpType
    PM = mybir.MatmulPerfMode
    f32, bf16, f8 = dt.float32, dt.bfloat16, dt.float8e4

    nc = bacc.Bacc("TRN2", target_bir_lowering=False, debug=False,
                   enable_asserts=False)

    def din(name, shape, d=f32):
        return nc.dram_tensor(name, list(shape), d, kind="ExternalInput").ap()

    x_d = din("x", (C, N))
    qwT_d = din("qwT", (128, NCP * C), f8)
    kwT_d = din("kwT", (128, NCP * C), f8)
    vwT_d = din("vwT", (128, NCP * C), f8)
    pwT_d = din("pwT", (128, NCP * C), f8)
    f1wT_d = din("f1wT", (128, NCP * HID), f8)
    f2wT_d = din("f2wT", (128, NH * C), f8)
    dwd_d = din("dwd", (128, NH * 9 * 128), f8)
    qb_d = din("qb8", (128, NC_))
    kb_d = din("kb8", (128, NC_))
    f1b_d = din("f1b", (128, NH))
    dwb_d = din("dwb", (128, NH))
    al1_d = din("al1_32", (128, NC_))
    resb_d = din("resb", (128, NC_))
    out_d = nc.dram_tensor("out", [C, N], f32, kind="ExternalOutput").ap()
    chain = [x_d]
    for i in range(1, iters):
        chain.append(nc.dram_tensor(f"mid{i}", [C, N], f32).ap())
    chain.append(out_d)

    with tile.TileContext(nc) as tc:
      for it in range(iters):
        x_d, out_d = chain[it], chain[it + 1]
        with ExitStack() as top:
          consts = top.enter_context(tc.tile_pool(name="consts", bufs=1))
          x1p = top.enter_context(tc.tile_pool(name="x1p", bufs=1))
          pmm = top.enter_context(tc.tile_pool(name="pmm", bufs=2, space="PSUM"))

          def load_const(ap, shape, d=f32, tag=None):
              t = consts.tile(list(shape), d, tag=tag, name=tag)
              nc.sync.dma_start(t[:], ap)
              return t

          x1_t = x1p.tile([128, NC_ * N], f32, tag="x1", name="x1")
          x1q_t = x1p.tile([128, NCP * N], f8, tag="x1q", name="x1q")
          nc.gpsimd.memset(x1q_t[:, NC_ * N:], 0.0)

          with ExitStack() as attn_scope:
              wq = attn_scope.enter_context(tc.tile_pool(name="wq", bufs=1))
              xp = attn_scope.enter_context(tc.tile_pool(name="xp", bufs=1))
              qkp = attn_scope.enter_context(tc.tile_pool(name="qkp", bufs=1))
              vTp = attn_scope.enter_context(tc.tile_pool(name="vTp", bufs=1))
              anp = attn_scope.enter_context(tc.tile_pool(name="anp", bufs=1))

              qwT_s = wq.tile([128, NCP * C], f8, tag="qw", name="qw")
              kwT_s = wq.tile([128, NCP * C], f8, tag="kw", name="kw")
              vwT_s = wq.tile([128, NCP * C], f8, tag="vw", name="vw")
              pwT_s = wq.tile([128, NCP * C], f8, tag="pw", name="pw")
              nc.sync.dma_start(qwT_s[:], qwT_d)
              nc.sync.dma_start(kwT_s[:], kwT_d)

              x_t = xp.tile([128, NC_ * N], f32, tag="x", name="x")
              xq_t = xp.tile([128, NCP * N], f8, tag="xq", name="xq")
              q_t = qkp.tile([128, NCP * N], f8, tag="q", name="q")
              k_t = qkp.tile([128, NCP * N], f8, tag="k", name="k")
              vT_t = vTp.tile([128, NMB * C], f8, tag="vT", name="vT")
              attn_t = anp.tile([128, NCP * N], f8, tag="attn", name="attn")
              # zero the pad chunks (DR pairs read them; 0 * w == 0)
              nc.vector.memset(xq_t[:, NC_ * N:], 0.0)
              nc.gpsimd.memset(q_t[:, NC_ * N:], 0.0)
              nc.gpsimd.memset(k_t[:, NC_ * N:], 0.0)
              nc.gpsimd.memset(attn_t[:, NC_ * N:], 0.0)

              qwv = qwT_s[:].rearrange("p (c m) -> p c m", c=NCP)
              kwv = kwT_s[:].rearrange("p (c m) -> p c m", c=NCP)
              vwv = vwT_s[:].rearrange("p (c m) -> p c m", c=NCP)
              pwv = pwT_s[:].rearrange("p (c m) -> p c m", c=NCP)
              xqv = xq_t[:].rearrange("p (c n) -> p c n", c=NCP)
              qv = q_t[:].rearrange("p (c n) -> p c n", c=NCP)
              kv = k_t[:].rearrange("p (c n) -> p c n", c=NCP)
              vTv = vT_t[:].rearrange("p (t c) -> p t c", t=NMB)
              anv = attn_t[:].rearrange("p (c n) -> p c n", c=NCP)

              qb_s = load_const(qb_d, (128, NC_), tag="qb")
              kb_s = load_const(kb_d, (128, NC_), tag="kb")
              al1_s = load_const(al1_d, (128, NC_), tag="al1")
              resb_s = load_const(resb_d, (128, NC_), tag="resb")
              f1b_s = load_const(f1b_d, (128, NH), tag="f1b")
              dwb_s = load_const(dwb_d, (128, NH), tag="dwb")
              ebias = consts.tile([128, 1], f32, tag="eb", name="eb")
              ones2 = consts.tile([128, 64], f8, tag="on2", name="on2")
              onesb = consts.tile([1, 128], bf16, tag="onb", name="onb")
              nc.gpsimd.memset(ebias[:], EBIAS)
              nc.gpsimd.memset(ones2[:], 1.0)
              nc.gpsimd.memset(onesb[:], 0.125)
              ones2v = ones2[:].rearrange("p (i m) -> p i m", i=2)

              # ---- load x, convert to fp8, q/k/vT ------------------------
              cvt = [nc.scalar.copy, nc.vector.tensor_copy, nc.gpsimd.tensor_copy]
              for ti, (n0, nn) in enumerate(NT5):
                  for c in range(NC_):
                      nc.sync.dma_start(x_t[:, c * N + n0:c * N + n0 + nn],
                                        x_d[c * 128:(c + 1) * 128, n0:n0 + nn])
                      cvt[c](xq_t[:, c * N + n0:c * N + n0 + nn],
                             x_t[:, c * N + n0:c * N + n0 + nn])
                  if ti == 0:
                      nc.sync.dma_start(vwT_s[:], vwT_d)
                      nc.sync.dma_start(pwT_s[:], pwT_d)
                  # q, k for this n-tile; PSUM evictions split DVE/ACT
                  for which, wv_, bt, dst in ((0, qwv, qb_s, q_t),
                                              (1, kwv, kb_s, k_t)):
                      for mc in range(NC_):
                          ps = pmm.tile([128, 512], f32, tag="mm", name="mm")
                          for j in range(2):
                              nc.tensor.matmul(
                                  ps[:, :nn],
                                  wv_[:, 2 * j:2 * j + 2, mc * 128:(mc + 1) * 128],
                                  xqv[:, 2 * j:2 * j + 2, n0:n0 + nn],
                                  start=(j == 0), stop=(j == 1),
                                  perf_mode=PM.DoubleRow)
                          if (which + mc) % 2 == 0:
                              nc.vector.tensor_scalar_add(
                                  dst[:, mc * N + n0:mc * N + n0 + nn],
                                  ps[:, :nn], bt[:, mc:mc + 1])
                          else:
                              nc.scalar.activation(
                                  dst[:, mc * N + n0:mc * N + n0 + nn],
                                  ps[:, :nn], AF.Identity,
                                  bias=bt[:, mc:mc + 1])
                  # vT for the 128-blocks of this n-tile
                  for nb in range(n0 // 128, (n0 + nn) // 128):
                      ps = pmm.tile([128, 512], f32, tag="mm", name="mm")
                      for j in range(2):
                          nc.tensor.matmul(
                              ps[:, :C],
                              xqv[:, 2 * j:2 * j + 2, nb * 128:(nb + 1) * 128],
                              vwv[:, 2 * j:2 * j + 2, :],
                              start=(j == 0), stop=(j == 1),
                              perf_mode=PM.DoubleRow)
                      if nb % 2 == 0:
                          nc.scalar.copy(vTv[:, nb, :], ps[:, :C])
                      else:
                          nc.vector.tensor_copy(vTv[:, nb, :], ps[:, :C])

              # ---- attention over n-tiles --------------------------------
              with ExitStack() as grp:
                  aTp = grp.enter_context(tc.tile_pool(name="aTp", bufs=2))
                  stp = grp.enter_context(tc.tile_pool(name="stp", bufs=2))
                  psc = grp.enter_context(
                      tc.tile_pool(name="psc", bufs=2, space="PSUM"))
                  pat = grp.enter_context(
                      tc.tile_pool(name="pat", bufs=2, space="PSUM"))
                  plp = grp.enter_context(
                      tc.tile_pool(name="plp", bufs=1, space="PSUM"))

                  for ti, (n0, nn) in enumerate(NT5):
                      aT = aTp.tile([128, NMB * 512], f8, tag="aT", name="aT")
                      aTv = aT[:].rearrange("p (t n) -> p t n", t=NMB)
                      for t in range(NMB):
                          ps = psc.tile([128, 512], f32, tag="sc", name="sc")
                          for j in range(2):
                              nc.tensor.matmul(
                                  ps[:, :nn],
                                  kv[:, 2 * j:2 * j + 2, t * 128:(t + 1) * 128],
                                  qv[:, 2 * j:2 * j + 2, n0:n0 + nn],
                                  start=(j == 0), stop=(j == 1),
                                  perf_mode=PM.DoubleRow)
                          nc.scalar.activation(aTv[:, t, :nn], ps[:, :nn],
                                               AF.Exp, scale=ESC,
                                               bias=ebias[:, 0:1])
                      # l[n] (rows 0..31 identical)
                      pl = plp.tile([128, 512], f32, tag="pl", name="pl")
                      for t in range(NMB // 2):
                          nc.tensor.matmul(
                              pl[0:32, :nn], ones2v,
                              aTv[:, 2 * t:2 * t + 2, :nn],
                              start=(t == 0), stop=(t == NMB // 2 - 1),
                              perf_mode=PM.DoubleRow)
                      # rb = broadcast(0.125 / l)
                      r32 = stp.tile([1, 512], f32, tag="r32", name="r32")
                      rb1 = stp.tile([1, 512], bf16, tag="rb1", name="rb1")
                      rbb = stp.tile([128, 512], bf16, tag="rbb", name="rbb")
                      nc.vector.reciprocal(r32[:, :nn], pl[0:1, :nn])
                      nc.vector.tensor_copy(rb1[:, :nn], r32[:, :nn])
                      pb = psc.tile([128, 512], f32, tag="sc", name="sc")
                      nc.tensor.matmul(pb[:, :nn], onesb[:], rb1[:, :nn],
                                       start=True, stop=True)
                      nc.vector.tensor_copy(rbb[:, :nn], pb[:, :nn])
                      # attn = (vT.T @ aT) * rb
                      for mc in range(NC_):
                          pa = pat.tile([128, 512], f32, tag="at", name="at")
                          for t in range(NMB // 2):
                              nc.tensor.matmul(
                                  pa[:, :nn],
                                  vTv[:, 2 * t:2 * t + 2, mc * 128:(mc + 1) * 128],
                                  aTv[:, 2 * t:2 * t + 2, :nn],
                                  start=(t == 0), stop=(t == NMB // 2 - 1),
                                  perf_mode=PM.DoubleRow)
                          nc.vector.tensor_tensor(
                              attn_t[:, mc * N + n0:mc * N + n0 + nn],
                              pa[:, :nn], rbb[:, :nn], op=ALU.mult)
                      # proj + residual: x1' = (32*al1) . x + (32*pw) @ attn
                      for mc in range(NC_):
                          ps = pmm.tile([128, 512], f32, tag="mm", name="mm")
                          for j in range(2):
                              nc.tensor.matmul(
                                  ps[:, :nn],
                                  pwv[:, 2 * j:2 * j + 2, mc * 128:(mc + 1) * 128],
                                  anv[:, 2 * j:2 * j + 2, n0:n0 + nn],
                                  start=(j == 0), stop=(j == 1),
                                  perf_mode=PM.DoubleRow)
                          nc.vector.scalar_tensor_tensor(
                              x1_t[:, mc * N + n0:mc * N + n0 + nn],
                              x_t[:, mc * N + n0:mc * N + n0 + nn],
                              al1_s[:, mc:mc + 1], ps[:, :nn],
                              op0=ALU.mult, op1=ALU.add)
                          nc.vector.tensor_copy(
                              x1q_t[:, mc * N + n0:mc * N + n0 + nn],
                              x1_t[:, mc * N + n0:mc * N + n0 + nn])

          # ---- MLP ---------------------------------------------------------
          with ExitStack() as mlp_scope:
              wm = mlp_scope.enter_context(tc.tile_pool(name="wm", bufs=1))
              hp = mlp_scope.enter_context(tc.tile_pool(name="hp", bufs=1))
              gp = mlp_scope.enter_context(tc.tile_pool(name="gp", bufs=2))
              outp = mlp_scope.enter_context(tc.tile_pool(name="outp", bufs=4))
              pdw = mlp_scope.enter_context(
                  tc.tile_pool(name="pdw", bufs=2, space="PSUM"))

              f1wT_s = wm.tile([128, NCP * HID], f8, tag="f1w", name="f1w")
              f2wT_s = wm.tile([128, NH * C], f8, tag="f2w", name="f2w")
              dwd_s = wm.tile([128, NH * 9 * 128], f8, tag="dwd", name="dwd")
              nc.sync.dma_start(f1wT_s[:], f1wT_d)
              nc.sync.dma_start(f2wT_s[:], f2wT_d)
              nc.sync.dma_start(dwd_s[:], dwd_d)
              f1wv = f1wT_s[:].rearrange("p (c m) -> p c m", c=NCP)
              f2wv = f2wT_s[:].rearrange("p (t m) -> p t m", t=NH)
              dwdv = dwd_s[:].rearrange("p (b m) -> p b m", m=128)
              x1qv = x1q_t[:].rearrange("p (c n) -> p c n", c=NCP)

              h_t = [hp.tile([128, PAD * PAD], f8, tag=f"h_{c}", name=f"h_{c}")
                     for c in range(NH)]
              for c in range(NH):
                  hv = h_t[c][:].rearrange("p (y x) -> p y x", y=PAD)
                  nc.gpsimd.memset(hv[:, 0, :], 0.0)
                  nc.gpsimd.memset(hv[:, PAD - 1, :], 0.0)
                  nc.gpsimd.memset(hv[:, :, 0], 0.0)
                  nc.gpsimd.memset(hv[:, :, PAD - 1], 0.0)

              # fc1 -> h (padded, fp8): h = ps * 2^-8 + f1b
              for ti, (n0, nn) in enumerate(NT6):
                  y0 = ti * ROWS_PER_TILE
                  for hc in range(NH):
                      ps = pmm.tile([128, 512], f32, tag="mm", name="mm")
                      for j in range(2):
                          nc.tensor.matmul(
                              ps[:, :nn],
                              f1wv[:, 2 * j:2 * j + 2, hc * 128:(hc + 1) * 128],
                              x1qv[:, 2 * j:2 * j + 2, n0:n0 + nn],
                              start=(j == 0), stop=(j == 1),
                              perf_mode=PM.DoubleRow)
                      dst = h_t[hc][:].rearrange(
                          "p (y x) -> p y x", y=PAD)[
                          :, y0 + 1:y0 + 1 + ROWS_PER_TILE, 1:1 + W]
                      psv = ps[:, :nn].rearrange("p (y x) -> p y x",
                                                 y=ROWS_PER_TILE)
                      if hc % 2 == 0:
                          nc.scalar.activation(dst, psv, AF.Identity,
                                               bias=f1b_s[:, hc:hc + 1],
                                               scale=2.0 ** -8)
                      else:
                          nc.vector.tensor_scalar(dst, psv, 2.0 ** -8,
                                                  f1b_s[:, hc:hc + 1],
                                                  op0=ALU.mult, op1=ALU.add)

              # dwconv (4 DR tap-pairs + 1 single) + gelu -> g ; fc2 + residual
              for ti, (n0, nn) in enumerate(NT6):
                  y0 = ti * ROWS_PER_TILE
                  g_t = gp.tile([128, NH * 384], f8, tag="g", name="g")
                  gv = g_t[:].rearrange("p (c n) -> p c n", c=NH)
                  for hc in range(NH):
                      hv = h_t[hc][:].rearrange("p (y x) -> p y x", y=PAD)
                      ps = pdw.tile([128, 512], f32, tag="dw", name="dw")
                      psv = ps[:, :nn].rearrange("p (y x) -> p y x",
                                                 y=ROWS_PER_TILE)
                      for pi, (t0, t1, stride) in enumerate(DW_PAIRS):
                          dy, dx = divmod(t0, 3)
                          rhs = _pairw(
                              hv[:, y0 + dy:y0 + dy + ROWS_PER_TILE,
                                 dx:dx + W], stride)
                          nc.tensor.matmul(
                              psv, dwdv[:, hc * 9 + 2 * pi:hc * 9 + 2 * pi + 2, :],
                              rhs, start=(pi == 0), stop=False,
                              perf_mode=PM.DoubleRow)
                      dy, dx = divmod(DW_SINGLE, 3)
                      nc.tensor.matmul(
                          psv, dwdv[:, hc * 9 + 8, :],
                          hv[:, y0 + dy:y0 + dy + ROWS_PER_TILE, dx:dx + W],
                          start=False, stop=True)
                      nc.scalar.activation(gv[:, hc, :nn], ps[:, :nn], AF.Gelu,
                                           bias=dwb_s[:, hc:hc + 1])
                  for mc in range(NC_):
                      ps = pmm.tile([128, 512], f32, tag="mm", name="mm")
                      for t in range(NH // 2):
                          nc.tensor.matmul(
                              ps[:, :nn],
                              f2wv[:, 2 * t:2 * t + 2, mc * 128:(mc + 1) * 128],
                              gv[:, 2 * t:2 * t + 2, :nn],
                              start=(t == 0), stop=(t == NH // 2 - 1),
                              perf_mode=PM.DoubleRow)
                      tmp = outp.tile([128, 384], f32, tag="tmp", name="tmp")
                      nc.scalar.activation(tmp[:, :nn], ps[:, :nn], AF.Identity,
                                           bias=resb_s[:, mc:mc + 1],
                                           scale=2.0 ** -6)
                      ot = outp.tile([128, 384], f32, tag="ot", name="ot")
                      nc.vector.scalar_tensor_tensor(
                          ot[:, :nn], x1_t[:, mc * N + n0:mc * N + n0 + nn],
                          2.0 ** -5, tmp[:, :nn], op0=ALU.mult, op1=ALU.add)
                      nc.sync.dma_start(out_d[mc * 128:(mc + 1) * 128,
                                              n0:n0 + nn], ot[:, :nn])

    nc.compile()
    return nc


def _fold_inputs(inputs):
    """Host-side weight folding. Returns (shared weight map, per-core x list)."""
    f = np.float32
    g = {k: np.asarray(v, f) for k, v in inputs.items()}
    s1 = g['bn1_g'] / np.sqrt(g['bn1_v'] + EPS)
    t1 = g['bn1_b'] - g['bn1_m'] * s1
    qw = g['q_w'] * s1[None, :]; qb = g['q_w'] @ t1 + g['q_b']
    kw = g['k_w'] * s1[None, :]; kb = g['k_w'] @ t1 + g['k_b']
    vw = g['v_w'] * s1[None, :]
    vb_eff = g['v_w'] @ t1 + g['v_b']
    ls1, ls2 = g['ls1'], g['ls2']
    pw = ls1[:, None] * g['po_w']
    alpha1 = 1.0 + ls1 * s1
    beta1 = ls1 * (g['po_b'] + t1) + pw @ vb_eff
    s2 = g['bn2_g'] / np.sqrt(g['bn2_v'] + EPS)
    t2 = g['bn2_b'] - g['bn2_m'] * s2
    f1w = g['fc1_w'] * s2[None, :]
    f1b = g['fc1_w'] @ t2 + g['fc1_b'] + f1w @ beta1
    f2w = ls2[:, None] * g['fc2_w']
    resb = beta1 + ls2 * g['fc2_b']

    def chunked_T(wm, scale, n_out):
        # [n_out, C_in] -> [128, NCP, n_out] fp8 with zero pad chunk
        ci = wm.shape[1]
        arr = np.zeros((128, NCP, n_out), f)
        arr[:, :ci // 128] = (scale * wm.T).reshape(ci // 128, 128, n_out
                                                    ).transpose(1, 0, 2)
        return np.ascontiguousarray(arr.reshape(128, NCP * n_out)).astype(F8)

    # dw diag blocks in DW_ORDER: [128, NH*9*128]
    dww = g['dw_w'].reshape(HID, 9)           # [hid, tap]
    dwd = np.zeros((128, NH, 9, 128), f)
    pidx = np.arange(128)
    for hc in range(NH):
        for bi, tap in enumerate(DW_ORDER):
            dwd[pidx, hc, bi, pidx] = dww[hc * 128:(hc + 1) * 128, tap]

    # f2wT: [128, NH, C] = f2w.T chunked over HID
    f2wT = (64.0 * f2w.T).reshape(NH, 128, C).transpose(1, 0, 2)

    w = {
        'qwT': chunked_T(qw, 8.0, C),
        'kwT': chunked_T(kw, 8.0, C),
        'vwT': chunked_T(vw, 8.0, C),
        'pwT': chunked_T(pw, 32.0, C),
        'f1wT': chunked_T(f1w, 8.0, HID),
        'f2wT': np.ascontiguousarray(f2wT.reshape(128, NH * C)).astype(F8),
        'dwd': np.ascontiguousarray(dwd.reshape(128, NH * 9 * 128)).astype(F8),
        'qb8': np.ascontiguousarray(8.0 * qb.reshape(NC_, 128).T),
        'kb8': np.ascontiguousarray(8.0 * kb.reshape(NC_, 128).T),
        'f1b': np.ascontiguousarray(f1b.reshape(NH, 128).T),
        'dwb': np.ascontiguousarray(g['dw_b'].reshape(NH, 128).T),
        'al1_32': np.ascontiguousarray(32.0 * alpha1.reshape(NC_, 128).T),
        'resb': np.ascontiguousarray(resb.reshape(NC_, 128).T),
    }
    xs = [np.ascontiguousarray(g['x'][b].reshape(C, N))
          for b in range(g['x'].shape[0])]
    return w, xs


def get_program():
    global _PROG
    if _PROG is None:
        _PROG = _build_program()
    return _PROG


def kernel(**inputs):
    from concourse.bass_utils import run_bass_kernel_spmd
    nc = get_program()
    w, xs = _fold_inputs(inputs)
    B = len(xs)
    in_maps = [{**w, 'x': xs[b]} for b in range(B)]
    res = run_bass_kernel_spmd(nc, in_maps, list(range(B)))
    out = np.stack([res.results[b]['out'].reshape(C, H, W) for b in range(B)])
    return out.astype(inputs['x'].dtype if hasattr(inputs['x'], 'dtype') else np.float32)
